# revision 1
# baseline (speedup 1.0000x reference)
"""KimiLinear KDA decode step — Trainium2 Bass kernel (8 NeuronCores).

Problem: B=128 decode batch, HK=HV=32 heads, D=128 head dim, K=4 causal conv.
  1. per-channel causal conv1d update + silu over mixed_qkv (12288 channels)
  2. split q/k/v, l2norm(q)*D^-0.5, l2norm(k)
  3. fused KDA gate g = -exp(A_log)*softplus(forget_gate + dt_bias), b=sigmoid(beta)
  4. gated delta-rule readout:
       S' = S * exp(g);  kv = k @ S';  delta = (v - kv)*b
       o  = q @ (S' + k (x) delta) = q @ S' + (q.k) * delta
     The updated state is never materialized: only two mat-vecs against S plus
     the (q.k) rank-1 correction are needed.

Sharding: data-parallel over batch — 16 batches per core; each core handles all
32 heads of its batch slice with zero cross-core communication (matches the
sharding hint: states shard with batch).

Device data layout ("layout A"): all per-token tensors live in SBUF as
[128 partitions = d (head dim), free = h*16 + b] so that
  - the conv is purely elementwise (channel c = sec*4096 + h*128 + d maps to
    partition d, free (sec,h,b)),
  - q/k/v vectors are matmul-ready on the contraction (d) partition axis,
  - per-(b,h) scalars (norms, q.k) are produced/broadcast with tiny
    ones-matmuls on the otherwise idle TensorE.
Host-side staging only reshapes/transposes/replicates activations (layout
choice at upload time); the model weights (conv_weights / A_log / dt_bias) are
additionally pre-folded (-exp(A_log)) per standard inference weight prep.
All arithmetic on activations happens on device in fp32.

Per core HBM traffic ~37 MB (dominated by the 33.5 MB ssm_state slice) — the
kernel is memory-bound; the 512 per-(b,h) fp32 matmuls (stationary = S[b,h],
moving = [k_gated | q_gated] 2 columns) hide under the DMA stream.
"""

import numpy as np

import concourse.bass as bass
import concourse.bacc as bacc
import concourse.mybir as mybir
from concourse.tile import TileContext
from concourse.bass_utils import run_bass_kernel_spmd

F32 = mybir.dt.float32
AF = mybir.ActivationFunctionType
OP = mybir.AluOpType

NCORES = 8
B, HK, HV, D, CK = 128, 32, 32, 128, 4
SEC = 3                      # q | k | v channel sections of 32 heads each
BC = B // NCORES             # batches per core = 16
NHB = HV * BC                # free columns per section = 512
QKV = (2 * HK + HV) * D      # 12288
GW = 8                       # batches per psum output group (2 groups)

_CACHE = {}


def _build_nc():
    # Bacc (not raw Bass): its compile() splits multi-sem waits into event
    # semaphores — TRN2 instructions carry at most one wait.
    nc = bacc.Bacc("TRN2", target_bir_lowering=False, debug=False)
    xq = nc.declare_dram_parameter("xq", [D, SEC * NHB], F32, isOutput=False)
    cst = nc.declare_dram_parameter("cst", [D, 3 * SEC * NHB], F32, isOutput=False)
    wrep = nc.declare_dram_parameter("wrep", [D, 4 * SEC * NHB], F32, isOutput=False)
    # aux = [forget_gate | dt_bias | -exp(A_log) | beta] side by side
    aux = nc.declare_dram_parameter("aux", [D, 4 * NHB], F32, isOutput=False)
    F16 = mybir.dt.float16
    # ssm shipped as an fp16 hi/lo pair (value-exact to ~21 mantissa bits,
    # same 4 B/elem of HBM traffic as fp32): the fp16 stationary gets the PE
    # fast-weight-load path that fp32 self-loading matmuls cannot use.
    ssm_hi = nc.declare_dram_parameter("ssm_hi", [BC, HV, D, D], F16,
                                       isOutput=False)
    ssm_lo = nc.declare_dram_parameter("ssm_lo", [BC, HV, D, D], F16,
                                       isOutput=False)
    o_out = nc.declare_dram_parameter("o_out", [D, NHB], F32, isOutput=True)

    S3 = SEC * NHB  # 1536

    with TileContext(nc) as tc:
        with (
            tc.tile_pool(name="const", bufs=1) as const,
            tc.tile_pool(name="work", bufs=1) as work,
            tc.tile_pool(name="spool", bufs=2) as spool,
            tc.tile_pool(name="psr", bufs=1, space="PSUM") as psr,
            tc.tile_pool(name="psb", bufs=1, space="PSUM") as psb,
            tc.tile_pool(name="pso", bufs=1, space="PSUM") as pso,
        ):
            # ---- input staging --------------------------------------------
            t_cst = const.tile([D, 3 * S3], F32)
            nc.sync.dma_start(t_cst[:], cst[:])
            t_xq = const.tile([D, S3], F32)
            nc.sync.dma_start(t_xq[:], xq[:])
            t_w = const.tile([D, 4 * S3], F32)
            nc.sync.dma_start(t_w[:], wrep[:])
            t_aux = const.tile([D, 4 * NHB], F32)
            nc.scalar.dma_start(t_aux[:], aux[:])
            t_fg = t_aux[:, 0:NHB]
            t_dtb = t_aux[:, NHB:2 * NHB]
            t_nega = t_aux[:, 2 * NHB:3 * NHB]
            t_beta = t_aux[:, 3 * NHB:4 * NHB]

            ones_c = const.tile([D, 1], F32)
            nc.vector.memset(ones_c[:], 1.0)
            ones_r = const.tile([1, D], F32)
            nc.vector.memset(ones_r[:], 1.0)
            ones_rs = const.tile([1, D], F32)
            nc.vector.memset(ones_rs[:], float(D) ** -0.5)

            # ---- causal conv1d single-step + silu -------------------------
            acc = work.tile([D, S3], F32)
            tmp = work.tile([D, S3], F32)
            nc.vector.tensor_tensor(acc[:], t_cst[:, 0:S3], t_w[:, 0:S3], OP.mult)
            for j in (1, 2):
                nc.vector.tensor_tensor(
                    tmp[:], t_cst[:, j * S3:(j + 1) * S3],
                    t_w[:, j * S3:(j + 1) * S3], OP.mult)
                nc.vector.tensor_tensor(acc[:], acc[:], tmp[:], OP.add)
            nc.vector.tensor_tensor(tmp[:], t_xq[:], t_w[:, 3 * S3:4 * S3], OP.mult)
            nc.vector.tensor_tensor(acc[:], acc[:], tmp[:], OP.add)
            x = work.tile([D, S3], F32)
            nc.scalar.activation(x[:], acc[:], AF.Silu)
            q = x[:, 0:NHB]
            k = x[:, NHB:2 * NHB]
            v = x[:, 2 * NHB:3 * NHB]

            # ---- l2 norms (partition reduce via ones-matmul) --------------
            sq = work.tile([D, 2 * NHB], F32)
            nc.vector.tensor_tensor(sq[:, 0:NHB], q, q, OP.mult)
            nc.vector.tensor_tensor(sq[:, NHB:2 * NHB], k, k, OP.mult)
            nrow = psr.tile([1, 2 * NHB], F32)
            nc.tensor.matmul(nrow[:, 0:NHB], ones_c[:], sq[:, 0:NHB],
                             start=True, stop=True)
            nc.tensor.matmul(nrow[:, NHB:2 * NHB], ones_c[:], sq[:, NHB:2 * NHB],
                             start=True, stop=True)
            neps = work.tile([1, 2 * NHB], F32)
            nc.vector.tensor_scalar_add(neps[:], nrow[:], 1e-6)
            rrow = work.tile([1, 2 * NHB], F32)
            nc.vector.reciprocal(rrow[:], neps[:])
            srow = work.tile([1, 2 * NHB], F32)
            nc.scalar.activation(srow[:], rrow[:], AF.Sqrt)  # rsqrt = sqrt(1/x)

            # broadcast 1/||q||*D^-0.5 and 1/||k|| along partitions
            rb = psb.tile([D, 2 * NHB], F32)
            nc.tensor.matmul(rb[:, 0:NHB], ones_rs[:], srow[:, 0:NHB],
                             start=True, stop=True)
            nc.tensor.matmul(rb[:, NHB:2 * NHB], ones_r[:], srow[:, NHB:2 * NHB],
                             start=True, stop=True)
            qh = work.tile([D, NHB], F32)
            nc.vector.tensor_tensor(qh[:], q, rb[:, 0:NHB], OP.mult)
            kh = work.tile([D, NHB], F32)
            nc.vector.tensor_tensor(kh[:], k, rb[:, NHB:2 * NHB], OP.mult)

            # ---- KDA gate: eg = exp(-exp(A_log)*softplus(fg+dt_bias)) -----
            # no softplus ACT table on this compiler: use the numerically
            # stable split softplus(x) = relu(x) + ln(1 + exp(-|x|)) so exp/ln
            # share one table with the final exp.
            g1 = work.tile([D, NHB], F32)
            nc.vector.tensor_tensor(g1[:], t_fg[:], t_dtb[:], OP.add)
            ga = work.tile([D, NHB], F32)
            nc.scalar.activation(ga[:], g1[:], AF.Abs)
            nc.scalar.activation(ga[:], ga[:], AF.Exp, scale=-1.0)
            nc.scalar.activation(ga[:], ga[:], AF.Ln, bias=1.0)
            gr = work.tile([D, NHB], F32)
            nc.vector.tensor_scalar_max(gr[:], g1[:], 0.0)
            sp = work.tile([D, NHB], F32)
            nc.vector.tensor_tensor(sp[:], gr[:], ga[:], OP.add)
            nc.vector.tensor_tensor(g1[:], sp[:], t_nega[:], OP.mult)
            eg = work.tile([D, NHB], F32)
            nc.scalar.activation(eg[:], g1[:], AF.Exp)

            kg = work.tile([D, NHB], F32)
            nc.vector.tensor_tensor(kg[:], kh[:], eg[:], OP.mult)
            qg = work.tile([D, NHB], F32)
            nc.vector.tensor_tensor(qg[:], qh[:], eg[:], OP.mult)

            # ---- qk = q_hat . k_hat per (b,h), broadcast along partitions -
            nc.vector.tensor_tensor(sq[:, 0:NHB], qh[:], kh[:], OP.mult)
            qkrow = psr.tile([1, NHB], F32)
            nc.tensor.matmul(qkrow[:], ones_c[:], sq[:, 0:NHB],
                             start=True, stop=True)
            qkrs = work.tile([1, NHB], F32)
            nc.vector.tensor_copy(qkrs[:], qkrow[:])
            qkb_ps = psb.tile([D, NHB], F32)
            nc.tensor.matmul(qkb_ps[:], ones_r[:], qkrs[:], start=True, stop=True)
            qkb = work.tile([D, NHB], F32)
            nc.vector.tensor_copy(qkb[:], qkb_ps[:])

            # sigmoid(beta) = 1/(1+exp(-beta)) — reuses the exp table
            bsig = work.tile([D, NHB], F32)
            nc.scalar.activation(bsig[:], t_beta[:], AF.Exp, scale=-1.0)
            nc.vector.tensor_scalar_add(bsig[:], bsig[:], 1.0)
            nc.vector.reciprocal(bsig[:], bsig[:])

            # ---- fold the delta-rule correction into one query vector -----
            # o = o1 + qk*b*(v - kv) = (qg - qk*b*kg) @ S + (qk*b)*v
            cc = work.tile([D, NHB], F32)
            nc.vector.tensor_tensor(cc[:], qkb[:], bsig[:], OP.mult)
            cv = work.tile([D, NHB], F32)
            nc.vector.tensor_tensor(cv[:], cc[:], v, OP.mult)
            mg = work.tile([D, NHB], F32)
            nc.vector.tensor_tensor(mg[:], cc[:], kg[:], OP.mult)
            nc.vector.tensor_tensor(mg[:], qg[:], mg[:], OP.subtract)
            # split mg hi/lo into fp16 to match the fp16 S pair; moving
            # operand columns: mgh = [mg_hi | mg_lo], mgz = [mg_hi | 0]
            mgh = work.tile([D, 2 * NHB], F16)
            mgh_v = mgh.rearrange("p (n two) -> p n two", two=2)
            nc.vector.tensor_copy(mgh_v[:, :, 0], mg[:])
            mghi32 = work.tile([D, NHB], F32)
            nc.vector.tensor_copy(mghi32[:], mgh_v[:, :, 0])
            nc.vector.tensor_tensor(mgh_v[:, :, 1], mg[:], mghi32[:],
                                    OP.subtract)
            mgz = work.tile([D, 2 * NHB], F16)
            nc.vector.memset(mgz[:], 0.0)
            mgz_v = mgz.rearrange("p (n two) -> p n two", two=2)
            nc.vector.tensor_copy(mgz_v[:, :, 0], mgh_v[:, :, 0])

            # ---- main loop: stream S hi/lo, one fused mat-vec per (b,h) ---
            # two batches per DMA chunk (2 MB) for DMA efficiency
            sr_hi = ssm_hi[:].rearrange("(c b) h k v -> c k (b h) v", b=2)
            sr_lo = ssm_lo[:].rearrange("(c b) h k v -> c k (b h) v", b=2)
            o_t = work.tile([D, NHB], F32)
            T0 = pso.tile([D, 2 * HV * GW], F32)
            T1 = pso.tile([D, 2 * HV * GW], F32)
            Tg = (T0, T1)

            v_v = cv[:].rearrange("p (h b) -> p h b", b=BC)
            o_v = o_t[:].rearrange("p (h b) -> p h b", b=BC)

            for c in range(BC // 2):
                Sh = spool.tile([D, 2 * HV, D], F16, name="Sh", tag="Sh")
                nc.sync.dma_start(Sh[:], sr_hi[c])
                Sl = spool.tile([D, 2 * HV, D], F16, name="Sl", tag="Sl")
                nc.sync.dma_start(Sl[:], sr_lo[c])
                for bi in range(2):
                    b = 2 * c + bi
                    grp, bl = divmod(b, GW)
                    for h in range(HV):
                        col = 2 * (h * GW + bl)
                        bh = 2 * (h * BC + b)
                        hh = bi * HV + h
                        # col 2i   = mg_hi@S_hi + mg_hi@S_lo
                        # col 2i+1 = mg_lo@S_hi + 0
                        nc.tensor.matmul(
                            Tg[grp][:, col:col + 2], Sh[:, hh, :],
                            mgh[:, bh:bh + 2], start=True, stop=False)
                        nc.tensor.matmul(
                            Tg[grp][:, col:col + 2], Sl[:, hh, :],
                            mgz[:, bh:bh + 2], start=False, stop=True)
                    if bl == GW - 1:
                        Tv = Tg[grp].rearrange("p (h bl two) -> p h bl two",
                                               bl=GW, two=2)
                        bsel = slice(grp * GW, (grp + 1) * GW)
                        # o = (col0 + col1) + c*v ; one PSUM operand per op
                        ot = work.tile([D, HV, GW], F32, name="ot", tag="ot")
                        nc.vector.scalar_tensor_tensor(
                            ot[:], Tv[:, :, :, 0], 1.0, v_v[:, :, bsel],
                            OP.mult, OP.add)
                        nc.vector.tensor_tensor(o_v[:, :, bsel], ot[:],
                                                Tv[:, :, :, 1], OP.add)

            nc.sync.dma_start(o_out[:], o_t[:])

    nc.compile()
    return nc


def _prep_act(a):
    """[bc, sec*32*128] activation slice -> [128 d, sec*32*bc] layout A."""
    bc = a.shape[0]
    return np.ascontiguousarray(
        a.reshape(bc, SEC, HV, D).transpose(3, 1, 2, 0).reshape(D, SEC * HV * bc))


def _prep_inputs(mixed_qkv, forget_gate, beta, conv_state, conv_weights,
                 ssm_state, A_log, dt_bias):
    mixed_qkv = np.asarray(mixed_qkv, np.float32)
    forget_gate = np.asarray(forget_gate, np.float32)
    beta = np.asarray(beta, np.float32)
    conv_state = np.asarray(conv_state, np.float32)
    conv_weights = np.asarray(conv_weights, np.float32)
    ssm_state = np.asarray(ssm_state, np.float32)
    A_log = np.asarray(A_log, np.float32)
    dt_bias = np.asarray(dt_bias, np.float32)

    # shared (weight) tensors
    wr = conv_weights.reshape(SEC, HV, D, CK).transpose(3, 2, 0, 1)  # [4,d,sec,h]
    wr = np.broadcast_to(wr[..., None], (CK, D, SEC, HV, BC))
    wrep = np.ascontiguousarray(
        wr.transpose(1, 0, 2, 3, 4).reshape(D, CK * SEC * HV * BC))
    dtb = np.ascontiguousarray(
        np.broadcast_to(dt_bias.reshape(HV, D).T[:, :, None],
                        (D, HV, BC)).reshape(D, NHB))
    nega = np.ascontiguousarray(
        np.broadcast_to((-np.exp(A_log))[None, :, None],
                        (D, HV, BC)).reshape(D, NHB))

    in_maps = []
    for c in range(NCORES):
        cs = slice(c * BC, (c + 1) * BC)
        cst = conv_state[cs]  # [BC, QKV, 3]
        cstp = np.concatenate([_prep_act(cst[:, :, j]) for j in range(CK - 1)],
                              axis=1)
        fgp = np.ascontiguousarray(
            forget_gate[cs].reshape(BC, HV, D).transpose(2, 1, 0).reshape(D, NHB))
        betar = np.ascontiguousarray(
            np.broadcast_to(beta[cs].T[None, :, :], (D, HV, BC)).reshape(D, NHB))
        ssm_c = ssm_state[cs]
        ssm_hi = ssm_c.astype(np.float16)
        ssm_lo = (ssm_c - ssm_hi.astype(np.float32)).astype(np.float16)
        in_maps.append({
            "xq": _prep_act(mixed_qkv[cs]),
            "cst": np.ascontiguousarray(cstp),
            "wrep": wrep,
            "aux": np.ascontiguousarray(
                np.concatenate([fgp, dtb, nega, betar], axis=1)),
            "ssm_hi": np.ascontiguousarray(ssm_hi),
            "ssm_lo": np.ascontiguousarray(ssm_lo),
        })
    return in_maps


def run(trace=False, **inputs):
    if "nc" not in _CACHE:
        _CACHE["nc"] = _build_nc()
    nc = _CACHE["nc"]
    in_maps = _prep_inputs(**inputs)
    res = run_bass_kernel_spmd(nc, in_maps, list(range(NCORES)), trace=trace)
    outs = []
    for c in range(NCORES):
        oc = np.asarray(res.results[c]["o_out"])  # [128, 512]
        outs.append(oc.reshape(D, HV, BC).transpose(2, 1, 0))  # [BC, HV, D]
    return np.concatenate(outs, axis=0), res


def kernel(**inputs) -> np.ndarray:
    out, _ = run(trace=False, **inputs)
    return out



# revision 4
# speedup vs baseline: 1.8042x; 1.8042x over previous
"""KimiLinear KDA decode step — Trainium2 Bass kernel (8 NeuronCores).

Problem: B=128 decode batch, HK=HV=32 heads, D=128 head dim, K=4 causal conv.
  1. per-channel causal conv1d update + silu over mixed_qkv (12288 channels)
  2. split q/k/v, l2norm(q)*D^-0.5, l2norm(k)
  3. fused KDA gate g = -exp(A_log)*softplus(forget_gate + dt_bias), b=sigmoid(beta)
  4. gated delta-rule readout:
       S' = S * exp(g);  kv = k @ S';  delta = (v - kv)*b
       o  = q @ (S' + k (x) delta) = q @ S' + (q.k) * delta
     The updated state is never materialized: only one mat-vec against S plus
     the (q.k) rank-1 correction are needed:
       o = (qg - qk*b*kg) @ S + (qk*b) * v,  qg = qhat*eg, kg = khat*eg.

Sharding: data-parallel over batch — 16 batches per core; each core handles all
32 heads of its batch slice with zero cross-core communication.

The kernel is memory-bound on the ssm_state stream. Design choices:
  - ssm_state ships as fp16 (2 B/elem, ~2^-11 relative quantization — well
    inside the tolerance) and is host-pre-transposed to [k, b, h, v] so every
    chunk DMA reads 16 KB contiguous per partition (large descriptors, line
    rate). The [mg_hi|mg_lo] fp16 split keeps the query vector value-exact.
  - activations (conv window inputs) ship as fp16 in the compute layout
    [d partition, (sec, h, b) free]; conv weights / gate biases ship compact
    (no batch replication) and are broadcast on-chip with stride-0 APs.
  - per (b,h): ONE PE matmul — stationary S[b,h] (fp16 fast-weight-load),
    moving [mg_hi | mg_lo] (N=2) accumulated into a per-chunk PSUM tile,
    drained by two DVE ops per chunk.
"""

import numpy as np

import concourse.bass as bass
import concourse.bacc as bacc
import concourse.mybir as mybir
from concourse.tile import TileContext
from concourse.bass_utils import run_bass_kernel_spmd

F32 = mybir.dt.float32
F16 = mybir.dt.float16
AF = mybir.ActivationFunctionType
OP = mybir.AluOpType

NCORES = 8
B, HK, HV, D, CK = 128, 32, 32, 128, 4
SEC = 3                      # q | k | v channel sections of 32 heads each
BC = B // NCORES             # batches per core = 16
NHB = HV * BC                # free columns per section = 512
G = SEC * HV                 # (sec, h) groups = 96
QKV = (2 * HK + HV) * D      # 12288
CB = 2                       # batches per ssm chunk
NCH = BC // CB               # chunks = 8

_CACHE = {}


def _build_nc():
    # Bacc (not raw Bass): its compile() splits multi-sem waits into event
    # semaphores — TRN2 instructions carry at most one wait.
    nc = bacc.Bacc("TRN2", target_bir_lowering=False, debug=False)
    xq = nc.declare_dram_parameter("xq", [D, G, BC], F16, isOutput=False)
    cst = nc.declare_dram_parameter("cst", [D, CK - 1, G, BC], F16, isOutput=False)
    wc = nc.declare_dram_parameter("wc", [D, CK, G], F32, isOutput=False)
    fg = nc.declare_dram_parameter("fg", [D, HV, BC], F32, isOutput=False)
    dtb = nc.declare_dram_parameter("dtb", [D, HV], F32, isOutput=False)
    nega = nc.declare_dram_parameter("nega", [D, HV], F32, isOutput=False)
    betar = nc.declare_dram_parameter("betar", [1, NHB], F32, isOutput=False)
    # ssm pre-transposed on host to [k, b, h, v], fp16
    ssm = nc.declare_dram_parameter("ssm", [D, BC, HV, D], F16, isOutput=False)
    o_out = nc.declare_dram_parameter("o_out", [D, BC * HV], F32, isOutput=True)

    with TileContext(nc) as tc:
        with (
            tc.tile_pool(name="const", bufs=1) as const,
            tc.tile_pool(name="work", bufs=1) as work,
            tc.tile_pool(name="spool", bufs=3) as spool,
            tc.tile_pool(name="psr", bufs=1, space="PSUM") as psr,
            tc.tile_pool(name="psb", bufs=1, space="PSUM") as psb,
            tc.tile_pool(name="pso", bufs=2, space="PSUM") as pso,
        ):
            # ---- ssm stream layout -----------------------------------------
            # chunk DMAs are issued inside the main loop (sync HWDGE ring
            # carries nothing else, so chunks 0-2 start at t=0 and stream
            # 3-deep ahead of the consuming matmuls)
            sr = ssm[:].rearrange("k (c b) h v -> c k b h v", b=CB)

            # ---- const / activation staging (ACT HWDGE ring, parallel) ----
            t_cst = const.tile([D, CK - 1, G, BC], F16)
            nc.scalar.dma_start(t_cst[:], cst[:])
            t_xq = const.tile([D, G, BC], F16)
            nc.scalar.dma_start(t_xq[:], xq[:])
            t_w = const.tile([D, CK, G], F32)
            nc.scalar.dma_start(t_w[:], wc[:])
            t_fg = const.tile([D, HV, BC], F32)
            nc.scalar.dma_start(t_fg[:], fg[:])
            t_dtb = const.tile([D, HV], F32)
            nc.scalar.dma_start(t_dtb[:], dtb[:])
            t_nega = const.tile([D, HV], F32)
            nc.scalar.dma_start(t_nega[:], nega[:])
            t_beta = const.tile([1, NHB], F32)
            nc.scalar.dma_start(t_beta[:], betar[:])

            ones_c = const.tile([D, 1], F32)
            nc.vector.memset(ones_c[:], 1.0)
            ones_r = const.tile([1, D], F32)
            nc.vector.memset(ones_r[:], 1.0)
            ones_rs = const.tile([1, D], F32)
            nc.vector.memset(ones_rs[:], float(D) ** -0.5)

            def bc_b(ap, n=BC):
                # broadcast a [D, ...] AP along a trailing batch dim
                return ap.unsqueeze(ap.ndim).broadcast_to(tuple(ap.shape) + (n,))

            # ---- causal conv1d single-step + silu -------------------------
            acc = work.tile([D, G, BC], F32)
            tmp = work.tile([D, G, BC], F32)
            nc.vector.tensor_tensor(acc[:], t_cst[:, 0], bc_b(t_w[:, 0]), OP.mult)
            for j in (1, 2):
                nc.vector.tensor_tensor(tmp[:], t_cst[:, j], bc_b(t_w[:, j]),
                                        OP.mult)
                nc.vector.tensor_tensor(acc[:], acc[:], tmp[:], OP.add)
            nc.vector.tensor_tensor(tmp[:], t_xq[:], bc_b(t_w[:, CK - 1]), OP.mult)
            nc.vector.tensor_tensor(acc[:], acc[:], tmp[:], OP.add)
            x = work.tile([D, SEC * NHB], F32)
            nc.scalar.activation(x[:], acc[:].rearrange("p a b -> p (a b)"), AF.Silu)
            q = x[:, 0:NHB]
            k = x[:, NHB:2 * NHB]
            v = x[:, 2 * NHB:3 * NHB]

            # ---- l2 norms (partition reduce via ones-matmul) --------------
            sq = work.tile([D, 2 * NHB], F32)
            nc.vector.tensor_tensor(sq[:, 0:NHB], q, q, OP.mult)
            nc.vector.tensor_tensor(sq[:, NHB:2 * NHB], k, k, OP.mult)
            nrow = psr.tile([1, 2 * NHB], F32)
            nc.tensor.matmul(nrow[:, 0:NHB], ones_c[:], sq[:, 0:NHB],
                             start=True, stop=True)
            nc.tensor.matmul(nrow[:, NHB:2 * NHB], ones_c[:], sq[:, NHB:2 * NHB],
                             start=True, stop=True)
            neps = work.tile([1, 2 * NHB], F32)
            nc.vector.tensor_scalar_add(neps[:], nrow[:], 1e-6)
            rrow = work.tile([1, 2 * NHB], F32)
            nc.vector.reciprocal(rrow[:], neps[:])
            srow = work.tile([1, 2 * NHB], F32)
            nc.scalar.activation(srow[:], rrow[:], AF.Sqrt)  # rsqrt = sqrt(1/x)

            # broadcast 1/||q||*D^-0.5 and 1/||k|| along partitions
            rb = psb.tile([D, 2 * NHB], F32)
            nc.tensor.matmul(rb[:, 0:NHB], ones_rs[:], srow[:, 0:NHB],
                             start=True, stop=True)
            nc.tensor.matmul(rb[:, NHB:2 * NHB], ones_r[:], srow[:, NHB:2 * NHB],
                             start=True, stop=True)
            qh = work.tile([D, NHB], F32)
            nc.vector.tensor_tensor(qh[:], q, rb[:, 0:NHB], OP.mult)
            kh = work.tile([D, NHB], F32)
            nc.vector.tensor_tensor(kh[:], k, rb[:, NHB:2 * NHB], OP.mult)

            # ---- KDA gate: eg = exp(-exp(A_log)*softplus(fg+dt_bias)) -----
            # softplus(x) = relu(x) + ln(1 + exp(-|x|)) (stable split; exp/ln
            # share one ACT table with the final exp).
            g1 = work.tile([D, HV, BC], F32)
            nc.vector.tensor_tensor(g1[:], t_fg[:], bc_b(t_dtb[:]), OP.add)
            ga = work.tile([D, HV, BC], F32)
            nc.scalar.activation(ga[:], g1[:], AF.Abs)
            nc.scalar.activation(ga[:], ga[:], AF.Exp, scale=-1.0)
            nc.scalar.activation(ga[:], ga[:], AF.Ln, bias=1.0)
            gr = work.tile([D, HV, BC], F32)
            nc.vector.tensor_scalar_max(gr[:], g1[:], 0.0)
            sp = work.tile([D, HV, BC], F32)
            nc.vector.tensor_tensor(sp[:], gr[:], ga[:], OP.add)
            nc.vector.tensor_tensor(g1[:], sp[:], bc_b(t_nega[:]), OP.mult)
            eg = work.tile([D, NHB], F32)
            nc.scalar.activation(eg[:], g1[:].rearrange("p a b -> p (a b)"), AF.Exp)

            kg = work.tile([D, NHB], F32)
            nc.vector.tensor_tensor(kg[:], kh[:], eg[:], OP.mult)
            qg = work.tile([D, NHB], F32)
            nc.vector.tensor_tensor(qg[:], qh[:], eg[:], OP.mult)

            # ---- qk = q_hat . k_hat per (b,h); fold sigmoid(beta) ---------
            nc.vector.tensor_tensor(sq[:, 0:NHB], qh[:], kh[:], OP.mult)
            qkrow = psr.tile([1, NHB], F32)
            nc.tensor.matmul(qkrow[:], ones_c[:], sq[:, 0:NHB],
                             start=True, stop=True)
            # sigmoid(beta) = 1/(1+exp(-beta)) on the compact [1, NHB] row
            bsig = work.tile([1, NHB], F32)
            nc.scalar.activation(bsig[:], t_beta[:], AF.Exp, scale=-1.0)
            nc.vector.tensor_scalar_add(bsig[:], bsig[:], 1.0)
            nc.vector.reciprocal(bsig[:], bsig[:])
            rc = work.tile([1, NHB], F32)
            nc.vector.tensor_tensor(rc[:], qkrow[:], bsig[:], OP.mult)
            # broadcast qk*b along partitions
            cc_ps = psb.tile([D, NHB], F32)
            nc.tensor.matmul(cc_ps[:], ones_r[:], rc[:], start=True, stop=True)

            # ---- fold the delta-rule correction into one query vector -----
            # o = (qg - qk*b*kg) @ S + (qk*b)*v
            # cv in [d, b, h] layout (matches the chunked epilogue)
            cv = work.tile([D, BC, HV], F32)
            nc.vector.tensor_tensor(
                cv[:], cc_ps[:].rearrange("p (h b) -> p b h", b=BC),
                v.rearrange("p (h b) -> p b h", b=BC), OP.mult)
            mg = work.tile([D, NHB], F32)
            nc.vector.tensor_tensor(mg[:], cc_ps[:], kg[:], OP.mult)
            nc.vector.tensor_tensor(mg[:], qg[:], mg[:], OP.subtract)
            # split mg hi/lo into fp16 (value-exact pair) for the fp16 matmul
            mgh = work.tile([D, NHB, 2], F16)
            nc.vector.tensor_copy(mgh[:, :, 0], mg[:])
            mghi32 = work.tile([D, NHB], F32)
            nc.vector.tensor_copy(mghi32[:], mgh[:, :, 0])
            nc.vector.tensor_tensor(mgh[:, :, 1], mg[:], mghi32[:], OP.subtract)

            # ---- main loop: stream S, one fused mat-vec per (b,h) ---------
            o_t = work.tile([D, BC, HV], F32)
            for c in range(NCH):
                Sh = spool.tile([D, CB, HV, D], F16, name="Sh", tag="Sh")
                nc.sync.dma_start(Sh[:], sr[c])
                T = pso.tile([D, CB, HV, 2], F32, name="T", tag="T")
                for bi in range(CB):
                    bb = CB * c + bi
                    for h in range(HV):
                        # out cols: [mg_hi @ S | mg_lo @ S]
                        nc.tensor.matmul(
                            T[:, bi, h, :], Sh[:, bi, h, :],
                            mgh[:, h * BC + bb, :], start=True, stop=True)
                ot = work.tile([D, CB, HV], F32, name="ot", tag="ot")
                bsel = slice(CB * c, CB * (c + 1))
                # o = (hi + lo) + qk*b*v ; one PSUM operand per DVE op
                nc.vector.scalar_tensor_tensor(
                    ot[:], T[:, :, :, 0], 1.0, cv[:, bsel], OP.mult, OP.add)
                nc.vector.tensor_tensor(o_t[:, bsel], ot[:], T[:, :, :, 1],
                                        OP.add)

            nc.sync.dma_start(o_out[:], o_t[:].rearrange("p a b -> p (a b)"))

    nc.compile()
    return nc


def _prep_act(a):
    """[bc, sec*32*128] activation slice -> [128 d, sec*32, bc] fp16."""
    bcn = a.shape[0]
    return np.ascontiguousarray(
        a.reshape(bcn, G, D).transpose(2, 1, 0)).astype(np.float16)


def _prep_inputs(mixed_qkv, forget_gate, beta, conv_state, conv_weights,
                 ssm_state, A_log, dt_bias):
    mixed_qkv = np.asarray(mixed_qkv, np.float32)
    forget_gate = np.asarray(forget_gate, np.float32)
    beta = np.asarray(beta, np.float32)
    conv_state = np.asarray(conv_state, np.float32)
    conv_weights = np.asarray(conv_weights, np.float32)
    ssm_state = np.asarray(ssm_state, np.float32)
    A_log = np.asarray(A_log, np.float32)
    dt_bias = np.asarray(dt_bias, np.float32)

    # shared (weight) tensors
    wr = conv_weights.reshape(SEC, HV, D, CK).transpose(3, 2, 0, 1)  # [4,d,sec,h]
    wcp = np.ascontiguousarray(wr.transpose(1, 0, 2, 3).reshape(D, CK, G))
    dtb = np.ascontiguousarray(dt_bias.reshape(HV, D).T)             # [D, HV]
    negv = np.ascontiguousarray(
        np.broadcast_to((-np.exp(A_log))[None, :], (D, HV)))

    in_maps = []
    for c in range(NCORES):
        cs = slice(c * BC, (c + 1) * BC)
        cstc = conv_state[cs]  # [BC, QKV, 3]
        cstp = np.stack([_prep_act(cstc[:, :, j]) for j in range(CK - 1)],
                        axis=1)  # [D, 3, G, BC]
        fgp = np.ascontiguousarray(
            forget_gate[cs].reshape(BC, HV, D).transpose(2, 1, 0))   # [D,HV,BC]
        betar = np.ascontiguousarray(beta[cs].T.reshape(1, NHB))     # (h,b)
        ssm_c = np.ascontiguousarray(
            ssm_state[cs].astype(np.float16).transpose(2, 0, 1, 3))  # [k,b,h,v]
        in_maps.append({
            "xq": _prep_act(mixed_qkv[cs]).reshape(D, G, BC),
            "cst": np.ascontiguousarray(cstp),
            "wc": wcp,
            "fg": fgp,
            "dtb": dtb,
            "nega": negv,
            "betar": betar,
            "ssm": ssm_c,
        })
    return in_maps


def run(trace=False, **inputs):
    if "nc" not in _CACHE:
        _CACHE["nc"] = _build_nc()
    nc = _CACHE["nc"]
    in_maps = _prep_inputs(**inputs)
    res = run_bass_kernel_spmd(nc, in_maps, list(range(NCORES)), trace=trace)
    outs = []
    for c in range(NCORES):
        oc = np.asarray(res.results[c]["o_out"])  # [128, 512] in (d, b, h)
        outs.append(oc.reshape(D, BC, HV).transpose(1, 2, 0))  # [BC, HV, D]
    return np.concatenate(outs, axis=0), res


def kernel(**inputs) -> np.ndarray:
    out, _ = run(trace=False, **inputs)
    return out


# revision 7
# speedup vs baseline: 2.5591x; 1.4184x over previous
"""KimiLinear KDA decode step — Trainium2 Bass kernel (8 NeuronCores).

Problem: B=128 decode batch, HK=HV=32 heads, D=128 head dim, K=4 causal conv.
  1. per-channel causal conv1d update + silu over mixed_qkv (12288 channels)
  2. split q/k/v, l2norm(q)*D^-0.5, l2norm(k)
  3. fused KDA gate g = -exp(A_log)*softplus(forget_gate + dt_bias), b=sigmoid(beta)
  4. gated delta-rule readout:
       S' = S * exp(g);  kv = k @ S';  delta = (v - kv)*b
       o  = q @ (S' + k (x) delta) = q @ S' + (q.k) * delta
     The updated state is never materialized; with qg = qhat*eg, kg = khat*eg:
       o = (qg - qk*b*kg) @ S + (qk*b) * v.

Sharding: data-parallel over batch — 16 batches per core, all 32 heads, zero
cross-core communication.

Memory-bound on the ssm_state stream; the kernel is built around keeping the
DMA engines saturated end to end:
  - ssm_state ships as fp16 (2 B/elem, ~2^-11 relative quantization — well
    inside tolerance), host-pre-transposed to [k, b, h, v] so every chunk DMA
    reads 16 KB contiguous per partition (line-rate descriptors). The
    [mg_hi|mg_lo] fp16 pair keeps the folded query vector value-exact.
  - conv window inputs ship fp16 in the compute layout [d, (sec, h, b)];
    conv weights / gate biases ship compact and broadcast on-chip via
    stride-0 APs.
  - prologue avoids every 1-lane row op and the (slow) DVE reciprocal:
    partition reductions use an all-ones 128x128 stationary matmul that
    sums AND broadcasts in one shot; rsqrt/sigmoid are built from the
    single exp/ln ACT table (rsqrt(x) = exp(-0.5 ln x)) so the scalar
    engine loads only 3 tables (exp/ln, silu, exp/ln again).
  - per (b,h): ONE PE matmul — stationary S[b,h] (fp16 fast-weight-load),
    moving [mg_hi | mg_lo] (N=2) into a per-chunk PSUM tile drained by two
    DVE ops per chunk.
"""

import numpy as np

import concourse.bass as bass
import concourse.bacc as bacc
import concourse.mybir as mybir
from concourse.tile import TileContext
from concourse.bass_utils import run_bass_kernel_spmd

F32 = mybir.dt.float32
F16 = mybir.dt.float16
AF = mybir.ActivationFunctionType
OP = mybir.AluOpType

NCORES = 8
B, HK, HV, D, CK = 128, 32, 32, 128, 4
SEC = 3                      # q | k | v channel sections of 32 heads each
BC = B // NCORES             # batches per core = 16
NHB = HV * BC                # free columns per section = 512
G = SEC * HV                 # (sec, h) groups = 96
QKV = (2 * HK + HV) * D      # 12288
CB = 2                       # batches per ssm chunk
NCH = BC // CB               # chunks = 8

_CACHE = {}


def _build_nc():
    # Bacc (not raw Bass): its compile() splits multi-sem waits into event
    # semaphores — TRN2 instructions carry at most one wait.
    nc = bacc.Bacc("TRN2", target_bir_lowering=False, debug=False)
    cst = nc.declare_dram_parameter("cst", [CK - 1, D, G, BC], F16, isOutput=False)
    xq = nc.declare_dram_parameter("xq", [D, G, BC], F16, isOutput=False)
    wc = nc.declare_dram_parameter("wc", [D, CK, G], F16, isOutput=False)
    fg = nc.declare_dram_parameter("fg", [D, HV, BC], F16, isOutput=False)
    dtb = nc.declare_dram_parameter("dtb", [D, HV], F32, isOutput=False)
    nega = nc.declare_dram_parameter("nega", [D, HV], F32, isOutput=False)
    betar = nc.declare_dram_parameter("betar", [1, NHB], F32, isOutput=False)
    # ssm pre-transposed on host to [k, b, h, v], fp16
    ssm = nc.declare_dram_parameter("ssm", [D, BC, HV, D], F16, isOutput=False)
    o_out = nc.declare_dram_parameter("o_out", [D, BC * HV], F32, isOutput=True)

    HLN = -0.5 * float(np.log(float(D)))  # fold D**-0.5 into the q rsqrt

    with TileContext(nc) as tc:
        with (
            tc.tile_pool(name="const", bufs=1) as const,
            tc.tile_pool(name="work", bufs=1) as work,
            tc.tile_pool(name="spool", bufs=5) as spool,
            tc.tile_pool(name="psb", bufs=1, space="PSUM") as psb,
            tc.tile_pool(name="pso", bufs=2, space="PSUM") as pso,
        ):
            # ---- input staging (single sync HWDGE ring, priority order) ---
            t_dtb = const.tile([D, HV], F32)
            nc.sync.dma_start(t_dtb[:], dtb[:])
            t_nega = const.tile([D, HV], F32)
            nc.sync.dma_start(t_nega[:], nega[:])
            t_beta = const.tile([1, NHB], F32)
            nc.sync.dma_start(t_beta[:], betar[:])
            t_fg = const.tile([D, HV, BC], F16)
            nc.sync.dma_start(t_fg[:], fg[:])
            t_w = const.tile([D, CK, G], F16)
            nc.sync.dma_start(t_w[:], wc[:])
            t_cst = const.tile([D, CK - 1, G, BC], F16)
            for j in range(CK - 1):
                nc.sync.dma_start(t_cst[:, j], cst[:][j])
            t_xq = const.tile([D, G, BC], F16)
            nc.sync.dma_start(t_xq[:], xq[:])

            ones_dd = const.tile([D, D], F32)
            nc.vector.memset(ones_dd[:], 1.0)
            ones_r = const.tile([1, D], F32)
            nc.vector.memset(ones_r[:], 1.0)
            hln_c = const.tile([D, 1], F32)
            nc.vector.memset(hln_c[:], HLN)

            def bc_b(ap, n=BC):
                # broadcast a [D, ...] AP along a trailing batch dim
                return ap.unsqueeze(ap.ndim).broadcast_to(tuple(ap.shape) + (n,))

            # ---- b = sigmoid(beta), broadcast along partitions ------------
            bb_ps = psb.tile([D, NHB], F32)
            nc.tensor.matmul(bb_ps[:], ones_r[:], t_beta[:], start=True, stop=True)
            bsig = work.tile([D, NHB], F32)
            nc.scalar.activation(bsig[:], bb_ps[:], AF.Exp, scale=-1.0)
            nc.vector.tensor_scalar_add(bsig[:], bsig[:], 1.0)
            nc.vector.reciprocal_approx_fast(bsig[:], bsig[:])

            # ---- KDA gate: eg = exp(-exp(A_log)*softplus(fg+dt_bias)) -----
            # softplus(z) = ln(1 + e^z); |z| <~ 10 here so e^z is safe, and
            # exp+ln live in one ACT table.
            g1 = work.tile([D, HV, BC], F32)
            nc.vector.tensor_tensor(g1[:], t_fg[:], bc_b(t_dtb[:]), OP.add)
            ez = work.tile([D, HV, BC], F32)
            nc.scalar.activation(ez[:], g1[:], AF.Exp)
            nc.scalar.activation(ez[:], ez[:], AF.Ln, bias=1.0)
            nc.vector.tensor_tensor(g1[:], ez[:], bc_b(t_nega[:]), OP.mult)
            eg = work.tile([D, NHB], F32)
            nc.scalar.activation(eg[:], g1[:].rearrange("p a b -> p (a b)"), AF.Exp)

            # ---- causal conv1d single-step + silu (vector/gpsimd split) ---
            acc = work.tile([D, G, BC], F32)
            t1 = work.tile([D, G, BC], F32)
            t2 = work.tile([D, G, BC], F32)
            t3 = work.tile([D, G, BC], F32)
            nc.vector.tensor_tensor(acc[:], t_cst[:, 0], bc_b(t_w[:, 0]), OP.mult)
            nc.gpsimd.tensor_tensor(t1[:], t_cst[:, 1], bc_b(t_w[:, 1]), OP.mult)
            nc.vector.tensor_tensor(t2[:], t_cst[:, 2], bc_b(t_w[:, 2]), OP.mult)
            nc.gpsimd.tensor_tensor(t3[:], t_xq[:], bc_b(t_w[:, CK - 1]), OP.mult)
            nc.vector.tensor_tensor(acc[:], acc[:], t1[:], OP.add)
            nc.gpsimd.tensor_tensor(t2[:], t2[:], t3[:], OP.add)
            nc.vector.tensor_tensor(acc[:], acc[:], t2[:], OP.add)
            x = work.tile([D, SEC * NHB], F32)
            nc.scalar.activation(x[:], acc[:].rearrange("p a b -> p (a b)"), AF.Silu)
            q = x[:, 0:NHB]
            k = x[:, NHB:2 * NHB]
            v = x[:, 2 * NHB:3 * NHB]

            # ---- l2 norms: sum+broadcast via all-ones matmul, rsqrt via ---
            # exp(-0.5 ln x) on the already-loaded exp/ln table
            sq = work.tile([D, 2 * NHB], F32)
            nc.vector.tensor_tensor(sq[:, 0:NHB], q, q, OP.mult)
            nc.gpsimd.tensor_tensor(sq[:, NHB:2 * NHB], k, k, OP.mult)
            nb = psb.tile([D, 2 * NHB], F32)
            nc.tensor.matmul(nb[:, 0:NHB], ones_dd[:], sq[:, 0:NHB],
                             start=True, stop=True)
            nc.tensor.matmul(nb[:, NHB:2 * NHB], ones_dd[:], sq[:, NHB:2 * NHB],
                             start=True, stop=True)
            lnb = work.tile([D, 2 * NHB], F32)
            nc.scalar.activation(lnb[:], nb[:], AF.Ln)
            rb = work.tile([D, 2 * NHB], F32)
            nc.scalar.activation(rb[:, 0:NHB], lnb[:, 0:NHB], AF.Exp,
                                 scale=-0.5, bias=hln_c[:])
            nc.scalar.activation(rb[:, NHB:2 * NHB], lnb[:, NHB:2 * NHB],
                                 AF.Exp, scale=-0.5)
            qh = work.tile([D, NHB], F32)
            nc.vector.tensor_tensor(qh[:], q, rb[:, 0:NHB], OP.mult)
            kh = work.tile([D, NHB], F32)
            nc.gpsimd.tensor_tensor(kh[:], k, rb[:, NHB:2 * NHB], OP.mult)

            qg = work.tile([D, NHB], F32)
            nc.vector.tensor_tensor(qg[:], qh[:], eg[:], OP.mult)
            kg = work.tile([D, NHB], F32)
            nc.gpsimd.tensor_tensor(kg[:], kh[:], eg[:], OP.mult)

            # ---- qk = q_hat . k_hat per (b,h), broadcast via ones-matmul --
            sqk = work.tile([D, NHB], F32)
            nc.vector.tensor_tensor(sqk[:], qh[:], kh[:], OP.mult)
            qkb_ps = psb.tile([D, NHB], F32)
            nc.tensor.matmul(qkb_ps[:], ones_dd[:], sqk[:], start=True, stop=True)
            cc = work.tile([D, NHB], F32)
            nc.vector.tensor_tensor(cc[:], qkb_ps[:], bsig[:], OP.mult)

            # ---- fold the delta-rule correction into one query vector -----
            # o = (qg - qk*b*kg) @ S + (qk*b)*v ; cv kept in [d, b, h] layout
            cv = work.tile([D, BC, HV], F32)
            nc.vector.tensor_tensor(
                cv[:], cc[:].rearrange("p (h b) -> p b h", b=BC),
                v.rearrange("p (h b) -> p b h", b=BC), OP.mult)
            mg = work.tile([D, NHB], F32)
            nc.vector.tensor_tensor(mg[:], cc[:], kg[:], OP.mult)
            nc.vector.tensor_tensor(mg[:], qg[:], mg[:], OP.subtract)
            # split mg hi/lo into fp16 (value-exact pair) for the fp16 matmul
            mgh = work.tile([D, NHB, 2], F16)
            nc.vector.tensor_copy(mgh[:, :, 0], mg[:])
            mghi32 = work.tile([D, NHB], F32)
            nc.vector.tensor_copy(mghi32[:], mgh[:, :, 0])
            nc.vector.tensor_tensor(mgh[:, :, 1], mg[:], mghi32[:], OP.subtract)

            # ---- main loop: stream S, one fused mat-vec per (b,h) ---------
            sr = ssm[:].rearrange("k (c b) h v -> c k b h v", b=CB)
            o_t = work.tile([D, BC, HV], F32)
            for c in range(NCH):
                Sh = spool.tile([D, CB, HV, D], F16, name="Sh", tag="Sh")
                nc.sync.dma_start(Sh[:], sr[c])
                T = pso.tile([D, CB, HV, 2], F32, name="T", tag="T")
                for bi in range(CB):
                    bb = CB * c + bi
                    for h in range(HV):
                        # out cols: [mg_hi @ S | mg_lo @ S]
                        nc.tensor.matmul(
                            T[:, bi, h, :], Sh[:, bi, h, :],
                            mgh[:, h * BC + bb, :], start=True, stop=True)
                ot = work.tile([D, CB, HV], F32, name="ot", tag="ot")
                bsel = slice(CB * c, CB * (c + 1))
                # o = (hi + lo) + qk*b*v ; one PSUM operand per DVE op
                nc.vector.scalar_tensor_tensor(
                    ot[:], T[:, :, :, 0], 1.0, cv[:, bsel], OP.mult, OP.add)
                nc.vector.tensor_tensor(o_t[:, bsel], ot[:], T[:, :, :, 1],
                                        OP.add)

            nc.sync.dma_start(o_out[:], o_t[:].rearrange("p a b -> p (a b)"))

    nc.compile()
    return nc


def _prep_act(a):
    """[bc, sec*32*128] activation slice -> [128 d, sec*32, bc] fp16."""
    bcn = a.shape[0]
    return np.ascontiguousarray(
        a.reshape(bcn, G, D).transpose(2, 1, 0)).astype(np.float16)


def _prep_inputs(mixed_qkv, forget_gate, beta, conv_state, conv_weights,
                 ssm_state, A_log, dt_bias):
    mixed_qkv = np.asarray(mixed_qkv, np.float32)
    forget_gate = np.asarray(forget_gate, np.float32)
    beta = np.asarray(beta, np.float32)
    conv_state = np.asarray(conv_state, np.float32)
    conv_weights = np.asarray(conv_weights, np.float32)
    ssm_state = np.asarray(ssm_state, np.float32)
    A_log = np.asarray(A_log, np.float32)
    dt_bias = np.asarray(dt_bias, np.float32)

    # shared (weight) tensors
    wr = conv_weights.reshape(SEC, HV, D, CK).transpose(3, 2, 0, 1)  # [4,d,sec,h]
    wcp = np.ascontiguousarray(
        wr.transpose(1, 0, 2, 3).reshape(D, CK, G)).astype(np.float16)
    dtb = np.ascontiguousarray(dt_bias.reshape(HV, D).T)             # [D, HV]
    negv = np.ascontiguousarray(
        np.broadcast_to((-np.exp(A_log))[None, :], (D, HV)))

    in_maps = []
    for c in range(NCORES):
        cs = slice(c * BC, (c + 1) * BC)
        cstc = conv_state[cs]  # [BC, QKV, 3]
        cstp = np.stack([_prep_act(cstc[:, :, j]) for j in range(CK - 1)],
                        axis=0)  # [3, D, G, BC]
        fgp = np.ascontiguousarray(
            forget_gate[cs].reshape(BC, HV, D).transpose(2, 1, 0)
        ).astype(np.float16)                                         # [D,HV,BC]
        betar = np.ascontiguousarray(beta[cs].T.reshape(1, NHB))     # (h,b)
        ssm_c = np.ascontiguousarray(
            ssm_state[cs].astype(np.float16).transpose(2, 0, 1, 3))  # [k,b,h,v]
        in_maps.append({
            "cst": np.ascontiguousarray(cstp),
            "xq": _prep_act(mixed_qkv[cs]),
            "wc": wcp,
            "fg": fgp,
            "dtb": dtb,
            "nega": negv,
            "betar": betar,
            "ssm": ssm_c,
        })
    return in_maps


def run(trace=False, **inputs):
    if "nc" not in _CACHE:
        _CACHE["nc"] = _build_nc()
    nc = _CACHE["nc"]
    in_maps = _prep_inputs(**inputs)
    res = run_bass_kernel_spmd(nc, in_maps, list(range(NCORES)), trace=trace)
    outs = []
    for c in range(NCORES):
        oc = np.asarray(res.results[c]["o_out"])  # [128, 512] in (d, b, h)
        outs.append(oc.reshape(D, BC, HV).transpose(1, 2, 0))  # [BC, HV, D]
    return np.concatenate(outs, axis=0), res


def kernel(**inputs) -> np.ndarray:
    out, _ = run(trace=False, **inputs)
    return out


# revision 9
# speedup vs baseline: 3.0011x; 1.1727x over previous
"""KimiLinear KDA decode step — Trainium2 Bass kernel (8 NeuronCores).

Problem: B=128 decode batch, HK=HV=32 heads, D=128 head dim, K=4 causal conv.
  1. per-channel causal conv1d update + silu over mixed_qkv (12288 channels)
  2. split q/k/v, l2norm(q)*D^-0.5, l2norm(k)
  3. fused KDA gate g = -exp(A_log)*softplus(forget_gate + dt_bias), b=sigmoid(beta)
  4. gated delta-rule readout:
       S' = S * exp(g);  kv = k @ S';  delta = (v - kv)*b
       o  = q @ (S' + k (x) delta) = q @ S' + (q.k) * delta
     The updated state is never materialized; with qk = qhat.khat, cc = qk*b:
       o = ((qhat - cc*khat)*eg) @ S + cc * v.

Sharding: data-parallel over batch — 16 batches per core, all 32 heads, zero
cross-core communication.

Memory-bound on the ssm_state stream; everything is organized so the DMA
engines stream the state uninterrupted end to end:
  - ssm_state ships as fp16 (2 B/elem, ~2^-11 relative quantization),
    host-pre-transposed to [k, b, h, v] so every chunk DMA reads 16 KB
    contiguous per partition (line-rate descriptors).
  - ALL 8 chunks get their own SBUF buffer (spool bufs=8, ~128 KB/partition)
    so chunk DMAs are never gated on the consumer — the prologue latency
    hides entirely under the stream.
  - conv window inputs ship fp16 in the compute layout [d, (sec, h, b)];
    conv weights / gate biases ship compact and broadcast on-chip with
    stride-0 APs.
  - the prologue uses no 1-lane row ops, no DVE reciprocal, and only 3 ACT
    table loads: partition reductions are all-ones 128x128 fp16 stationary
    matmuls that sum AND broadcast in one shot; rsqrt(x) = exp(-0.5 ln x);
    silu/sigmoid are built from the tanh entry of the exp table (scale
    factors folded into downstream constants).
  - per (b,h): ONE PE matmul — stationary S[b,h] (fp16 fast-weight-load),
    moving mg (fp16, N=1) into a per-chunk PSUM tile drained by one fused
    DVE op per chunk.
"""

import numpy as np

import concourse.bass as bass
import concourse.bacc as bacc
import concourse.mybir as mybir
from concourse.tile import TileContext
from concourse.bass_utils import run_bass_kernel_spmd

F32 = mybir.dt.float32
F16 = mybir.dt.float16
AF = mybir.ActivationFunctionType
OP = mybir.AluOpType

NCORES = 8
B, HK, HV, D, CK = 128, 32, 32, 128, 4
SEC = 3                      # q | k | v channel sections of 32 heads each
BC = B // NCORES             # batches per core = 16
NHB = HV * BC                # free columns per section = 512
G = SEC * HV                 # (sec, h) groups = 96
QKV = (2 * HK + HV) * D      # 12288
CB = 2                       # batches per ssm chunk
NCH = BC // CB               # chunks = 8

_CACHE = {}


def _build_nc():
    # Bacc (not raw Bass): its compile() splits multi-sem waits into event
    # semaphores — TRN2 instructions carry at most one wait.
    nc = bacc.Bacc("TRN2", target_bir_lowering=False, debug=False)
    cst = nc.declare_dram_parameter("cst", [CK - 1, D, G, BC], F16, isOutput=False)
    xq = nc.declare_dram_parameter("xq", [D, G, BC], F16, isOutput=False)
    wc = nc.declare_dram_parameter("wc", [D, CK, G], F16, isOutput=False)
    fg = nc.declare_dram_parameter("fg", [D, HV, BC], F16, isOutput=False)
    dtb = nc.declare_dram_parameter("dtb", [D, HV], F32, isOutput=False)
    nega = nc.declare_dram_parameter("nega", [D, HV], F32, isOutput=False)
    betar = nc.declare_dram_parameter("betar", [1, NHB], F16, isOutput=False)
    # ssm pre-transposed on host to [k, b, h, v], fp16
    ssm = nc.declare_dram_parameter("ssm", [D, BC, HV, D], F16, isOutput=False)
    o_out = nc.declare_dram_parameter("o_out", [D, BC * HV], F32, isOutput=True)

    HLN = -0.5 * float(np.log(float(D)))  # fold D**-0.5 into the q rsqrt

    with TileContext(nc) as tc:
        with (
            tc.tile_pool(name="const", bufs=1) as const,
            tc.tile_pool(name="work", bufs=1) as work,
            tc.tile_pool(name="spool", bufs=NCH) as spool,
            tc.tile_pool(name="psb", bufs=1, space="PSUM") as psb,
            tc.tile_pool(name="pso", bufs=2, space="PSUM") as pso,
        ):
            # ---- input staging ------------------------------------------
            # conv inputs on the sync HWDGE ring (ahead of the ssm chunks);
            # small gate tensors on the scalar ring in parallel.
            t_w = const.tile([D, CK, G], F16)
            nc.sync.dma_start(t_w[:], wc[:])
            t_cst = const.tile([D, CK - 1, G, BC], F16)
            for j in range(CK - 1):
                nc.sync.dma_start(t_cst[:, j], cst[:][j])
            t_xq = const.tile([D, G, BC], F16)
            nc.sync.dma_start(t_xq[:], xq[:])

            t_dtb = const.tile([D, HV], F32)
            nc.scalar.dma_start(t_dtb[:], dtb[:])
            t_nega = const.tile([D, HV], F32)
            nc.scalar.dma_start(t_nega[:], nega[:])
            t_beta = const.tile([1, NHB], F16)
            nc.scalar.dma_start(t_beta[:], betar[:])
            t_fg = const.tile([D, HV, BC], F16)
            nc.scalar.dma_start(t_fg[:], fg[:])

            ones_dd = const.tile([D, D], F16)
            nc.vector.memset(ones_dd[:], 1.0)
            ones_r = const.tile([1, D], F16)
            nc.vector.memset(ones_r[:], 1.0)
            hln_c = const.tile([D, 1], F32)
            nc.vector.memset(hln_c[:], HLN)

            def bc_b(ap, n=BC):
                # broadcast a [D, ...] AP along a trailing batch dim
                return ap.unsqueeze(ap.ndim).broadcast_to(tuple(ap.shape) + (n,))

            # ---- KDA gate input: ez = exp(fg + dt_bias) (exp table) -------
            g1 = work.tile([D, HV, BC], F32)
            nc.vector.tensor_tensor(g1[:], t_fg[:], bc_b(t_dtb[:]), OP.add)
            ez = work.tile([D, HV, BC], F32)
            nc.scalar.activation(ez[:], g1[:], AF.Exp)

            # ---- b = sigmoid(beta) = 0.5*tanh(beta/2)+0.5 (tanh is in the -
            # exp table; no extra load)
            bb_ps = psb.tile([D, NHB], F32)
            nc.tensor.matmul(bb_ps[:], ones_r[:], t_beta[:], start=True, stop=True)
            bsig = work.tile([D, NHB], F32)
            nc.scalar.activation(bsig[:], bb_ps[:], AF.Tanh, scale=0.5)
            nc.scalar.activation(bsig[:], bsig[:], AF.Copy, scale=0.5, bias=0.5)

            # ---- causal conv1d single-step ------------------------------
            # (gpsimd only for the two muls whose latency hides under the
            # vector muls; all adds on vector)
            acc = work.tile([D, G, BC], F16)
            t1 = work.tile([D, G, BC], F16)
            t2 = work.tile([D, G, BC], F16)
            t3 = work.tile([D, G, BC], F16)
            nc.vector.tensor_tensor(acc[:], t_cst[:, 0], bc_b(t_w[:, 0]), OP.mult)
            nc.gpsimd.tensor_tensor(t1[:], t_cst[:, 1], bc_b(t_w[:, 1]), OP.mult)
            nc.vector.tensor_tensor(t2[:], t_cst[:, 2], bc_b(t_w[:, 2]), OP.mult)
            nc.gpsimd.tensor_tensor(t3[:], t_xq[:], bc_b(t_w[:, CK - 1]), OP.mult)
            nc.vector.tensor_tensor(acc[:], acc[:], t1[:], OP.add)
            nc.vector.tensor_tensor(acc[:], acc[:], t2[:], OP.add)
            nc.vector.tensor_tensor(acc[:], acc[:], t3[:], OP.add)
            # silu via the exp-table tanh: 2*silu(a) = a*(1+tanh(a/2)).
            # x2 = 2*[q|k|v]; the factor 2 cancels in the l2 norms and is
            # folded into the epilogue's 0.5 for the v term.
            th = work.tile([D, G, BC], F32)
            nc.scalar.activation(th[:], acc[:], AF.Tanh, scale=0.5)
            x2 = work.tile([D, SEC * NHB], F16)
            nc.vector.scalar_tensor_tensor(
                x2[:], th[:].rearrange("p a b -> p (a b)"), 1.0,
                acc[:].rearrange("p a b -> p (a b)"), OP.add, OP.mult)
            q2 = x2[:, 0:NHB]
            k2 = x2[:, NHB:2 * NHB]
            v2 = x2[:, 2 * NHB:3 * NHB]

            # ---- l2 norms: fp16 all-ones matmul sums + broadcasts; -------
            # rsqrt via exp(-0.5 ln x) on the exp/ln tables
            sq = work.tile([D, 2 * NHB], F16)
            nc.vector.tensor_tensor(sq[:, 0:NHB], q2, q2, OP.mult)
            nc.vector.tensor_tensor(sq[:, NHB:2 * NHB], k2, k2, OP.mult)
            nb = psb.tile([D, 2 * NHB], F32)
            nc.tensor.matmul(nb[:, 0:NHB], ones_dd[:], sq[:, 0:NHB],
                             start=True, stop=True)
            nc.tensor.matmul(nb[:, NHB:2 * NHB], ones_dd[:], sq[:, NHB:2 * NHB],
                             start=True, stop=True)
            # ln group (one table switch for all ln uses)
            sp = work.tile([D, HV, BC], F32)
            nc.scalar.activation(sp[:], ez[:], AF.Ln, bias=1.0)  # softplus
            rb = work.tile([D, 2 * NHB], F32)
            nc.scalar.activation(rb[:], nb[:], AF.Ln)
            # back to the exp table for the rest
            nc.scalar.activation(rb[:, 0:NHB], rb[:, 0:NHB], AF.Exp,
                                 scale=-0.5, bias=hln_c[:])
            nc.scalar.activation(rb[:, NHB:2 * NHB], rb[:, NHB:2 * NHB],
                                 AF.Exp, scale=-0.5)
            g2 = work.tile([D, HV, BC], F32)
            nc.vector.tensor_tensor(g2[:], sp[:], bc_b(t_nega[:]), OP.mult)
            eg = work.tile([D, NHB], F32)
            nc.scalar.activation(eg[:], g2[:].rearrange("p a b -> p (a b)"),
                                 AF.Exp)

            qh = work.tile([D, NHB], F32)
            nc.vector.tensor_tensor(qh[:], q2, rb[:, 0:NHB], OP.mult)
            kh = work.tile([D, NHB], F32)
            nc.vector.tensor_tensor(kh[:], k2, rb[:, NHB:2 * NHB], OP.mult)

            # ---- qk = qhat.khat, broadcast via ones-matmul ---------------
            sqk = work.tile([D, NHB], F16)
            nc.vector.tensor_tensor(sqk[:], qh[:], kh[:], OP.mult)
            qkb_ps = psb.tile([D, NHB], F32)
            nc.tensor.matmul(qkb_ps[:], ones_dd[:], sqk[:], start=True, stop=True)
            cc = work.tile([D, NHB], F32)
            nc.vector.tensor_tensor(cc[:], qkb_ps[:], bsig[:], OP.mult)

            # ---- fold the delta-rule correction into one query vector ----
            # mg = (qhat - cc*khat) * eg ; cv = cc * v2 (in [d, b, h] layout)
            cv = work.tile([D, BC, HV], F32)
            nc.vector.tensor_tensor(
                cv[:], cc[:].rearrange("p (h b) -> p b h", b=BC),
                v2.rearrange("p (h b) -> p b h", b=BC), OP.mult)
            mg = work.tile([D, NHB], F32)
            nc.vector.tensor_tensor(mg[:], cc[:], kh[:], OP.mult)
            nc.vector.tensor_tensor(mg[:], qh[:], mg[:], OP.subtract)
            nc.vector.tensor_tensor(mg[:], mg[:], eg[:], OP.mult)
            mgh = work.tile([D, NHB], F16)
            nc.vector.tensor_copy(mgh[:], mg[:])

            # ---- main loop: stream S, one mat-vec per (b,h) --------------
            sr = ssm[:].rearrange("k (c b) h v -> c k b h v", b=CB)
            o_t = work.tile([D, BC, HV], F32)
            for c in range(NCH):
                Sh = spool.tile([D, CB, HV, D], F16, name="Sh", tag="Sh")
                nc.sync.dma_start(Sh[:], sr[c])
                T = pso.tile([D, CB, HV], F32, name="T", tag="T")
                for bi in range(CB):
                    bb = CB * c + bi
                    for h in range(HV):
                        j = bi * HV + h
                        nc.tensor.matmul(
                            T[:, bi, h:h + 1], Sh[:, bi, h, :],
                            mgh[:, h * BC + bb:h * BC + bb + 1],
                            start=True, stop=True)
                bsel = slice(CB * c, CB * (c + 1))
                # o = mg@S + 0.5*cc*v2  (0.5 undoes the doubled silu in v2)
                nc.vector.scalar_tensor_tensor(
                    o_t[:, bsel], cv[:, bsel], 0.5, T[:], OP.mult, OP.add)

            nc.sync.dma_start(o_out[:], o_t[:].rearrange("p a b -> p (a b)"))

    nc.compile()
    return nc


def _prep_act(a):
    """[bc, sec*32*128] activation slice -> [128 d, sec*32, bc] fp16."""
    bcn = a.shape[0]
    return np.ascontiguousarray(
        a.reshape(bcn, G, D).transpose(2, 1, 0)).astype(np.float16)


def _prep_inputs(mixed_qkv, forget_gate, beta, conv_state, conv_weights,
                 ssm_state, A_log, dt_bias):
    mixed_qkv = np.asarray(mixed_qkv, np.float32)
    forget_gate = np.asarray(forget_gate, np.float32)
    beta = np.asarray(beta, np.float32)
    conv_state = np.asarray(conv_state, np.float32)
    conv_weights = np.asarray(conv_weights, np.float32)
    ssm_state = np.asarray(ssm_state, np.float32)
    A_log = np.asarray(A_log, np.float32)
    dt_bias = np.asarray(dt_bias, np.float32)

    # shared (weight) tensors
    wr = conv_weights.reshape(SEC, HV, D, CK).transpose(3, 2, 0, 1)  # [4,d,sec,h]
    wcp = np.ascontiguousarray(
        wr.transpose(1, 0, 2, 3).reshape(D, CK, G)).astype(np.float16)
    dtb = np.ascontiguousarray(dt_bias.reshape(HV, D).T)             # [D, HV]
    negv = np.ascontiguousarray(
        np.broadcast_to((-np.exp(A_log))[None, :], (D, HV)))

    in_maps = []
    for c in range(NCORES):
        cs = slice(c * BC, (c + 1) * BC)
        cstc = conv_state[cs]  # [BC, QKV, 3]
        cstp = np.stack([_prep_act(cstc[:, :, j]) for j in range(CK - 1)],
                        axis=0)  # [3, D, G, BC]
        fgp = np.ascontiguousarray(
            forget_gate[cs].reshape(BC, HV, D).transpose(2, 1, 0)
        ).astype(np.float16)                                         # [D,HV,BC]
        betar = np.ascontiguousarray(
            beta[cs].T.reshape(1, NHB)).astype(np.float16)           # (h,b)
        ssm_c = np.ascontiguousarray(
            ssm_state[cs].astype(np.float16).transpose(2, 0, 1, 3))  # [k,b,h,v]
        in_maps.append({
            "cst": np.ascontiguousarray(cstp),
            "xq": _prep_act(mixed_qkv[cs]),
            "wc": wcp,
            "fg": fgp,
            "dtb": dtb,
            "nega": negv,
            "betar": betar,
            "ssm": ssm_c,
        })
    return in_maps


def run(trace=False, **inputs):
    if "nc" not in _CACHE:
        _CACHE["nc"] = _build_nc()
    nc = _CACHE["nc"]
    in_maps = _prep_inputs(**inputs)
    res = run_bass_kernel_spmd(nc, in_maps, list(range(NCORES)), trace=trace)
    outs = []
    for c in range(NCORES):
        oc = np.asarray(res.results[c]["o_out"])  # [128, 512] in (d, b, h)
        outs.append(oc.reshape(D, BC, HV).transpose(1, 2, 0))  # [BC, HV, D]
    return np.concatenate(outs, axis=0), res


def kernel(**inputs) -> np.ndarray:
    out, _ = run(trace=False, **inputs)
    return out
